# revision 1
# baseline (speedup 1.0000x reference)
"""RBF-kernel attention (nn_Attention_76081050682051) on 8 TRN2 NeuronCores.

Self-contained Bass/Tile kernel. `kernel(**inputs)` takes the FULL unsharded
inputs of reference.setup_inputs() and returns the FULL [4, 2048, 256] f32
output.

Sharding (B x tensor-parallel heads): core c -> batch b = c//2, heads
[4*(c%2), 4*(c%2)+4); pairwise AllReduce ([0,1],[2,3],[4,5],[6,7]) combines
the two half-head partial outputs of each batch after the W_o projection.

Device math (f32r matmuls = 11-bit-mantissa fp32 at full PE rate):
  x [S, E] loaded untransposed; LayerNorm stats per-partition via
  bn_stats/bn_aggr; rsqrt via DVE reciprocal + Newton (ACT runs exp only ->
  a single activation-table load); xnT blocks produced by PE transposes.
  Per head: K'T/Q'T = (folded W).T @ xnT with sqrt(2*gamma)*ln_scale folded
  into W_q/W_k on the host; V = xnT.T-slices @ W_v.
  scoresT[t, s] = exp(qk'[t,s] - k2'[t]/2) via one ACT op per [128,512] tile
  (per-partition bias); the exp(-q2'[s]/2) factor is applied after W_o as a
  per-partition scale, so no broadcast over the S x S matrix is needed.
  outT = V.T @ scoresT accumulates over t in PSUM; W_o runs on outT column
  slices; partial outputs AllReduce within each batch pair.
  Emission is software-pipelined across heads (next head's projections are
  front-loaded into the current head's score loop) because pool-slot grants
  are FIFO in emission order.
"""
import sys
sys.path.insert(0, '/opt/trn_rl_repo')
import numpy as np
from concourse import bass, bacc, tile, mybir, masks
from concourse.bass_utils import run_bass_kernel_spmd

F32 = mybir.dt.float32
F32R = mybir.dt.float32r
AF = mybir.ActivationFunctionType
OP = mybir.AluOpType

B, S, E, H = 4, 2048, 256, 8
HL = 4          # heads per core
EC = 2          # e chunks of 128
SB = 4          # s blocks of 512
ST = 16         # s/t tiles of 128
N_CORES = 8
EPS = 1e-5

NO_COLL = False
N_HEADS_BUILD = HL


def build_kernel(R=1, debug=False):
    nc = bacc.Bacc("TRN2", target_bir_lowering=False, debug=False,
                   num_devices=N_CORES)

    x_ext = nc.declare_dram_parameter("x", [S, E], F32, isOutput=False)
    w_ext = {}
    for wname in ("wq", "wk", "wv", "wo"):
        # host pre-lays out as [head, partition, ec*e] so the per-head load
        # is one contiguous 2-D DMA (HWDGE, no SWDGE descriptor generation)
        w_ext[wname] = nc.declare_dram_parameter(wname, [HL, 128, EC * E], F32,
                                                 isOutput=False)
    out_ext = nc.declare_dram_parameter("out", [S, E], F32, isOutput=True)
    dbg_ext = {}
    if debug:
        dbg_ext['xn'] = nc.declare_dram_parameter("dbg_xn", [E, S], F32, isOutput=True)
        dbg_ext['qt'] = nc.declare_dram_parameter("dbg_qt", [E, S], F32, isOutput=True)
        dbg_ext['v'] = nc.declare_dram_parameter("dbg_v", [128, ST * E], F32, isOutput=True)
        dbg_ext['q2'] = nc.declare_dram_parameter("dbg_q2", [128, ST], F32, isOutput=True)
        dbg_ext['part'] = nc.declare_dram_parameter("dbg_part", [128, ST * E], F32, isOutput=True)

    with tile.TileContext(nc) as tc:
        with tc.tile_pool(name="sb", bufs=1) as sb, \
             tc.tile_pool(name="sbt", bufs=1) as sbt, \
             tc.tile_pool(name="ps", bufs=1, space="PSUM") as ps, \
             tc.tile_pool(name="dram", bufs=1, space="DRAM") as dram:

            # ---------- constants ----------
            ones_col32 = sb.tile([128, 1], F32, name="ones_col32")
            nc.any.memset(ones_col32[:], 1.0)
            ones_col = sb.tile([128, 1], F32R, name="ones_col")
            nc.vector.tensor_copy(ones_col[:], ones_col32[:])
            ident16 = sb.tile([16, 16], F32, name="ident16")
            masks.make_identity(nc, ident16[:])
            ident128 = sb.tile([128, 128], F32, name="ident128")
            masks.make_identity(nc, ident128[:])

            # ---------- load x blocks first (sync queue) ----------
            xu_tiles = []
            for sbk in range(SB):
                xu = sbt.tile([128, 4 * E], F32, name="xu", tag="xu", bufs=4)
                nc.sync.dma_start(
                    xu[:].rearrange("p (t e) -> p t e", t=4),
                    x_ext[sbk * 512:(sbk + 1) * 512, :]
                    .rearrange("(t p) e -> p t e", p=128))
                xu_tiles.append(xu)

            pools = dict(sb=sb, sbt=sbt, ps=ps, dram=dram)
            _build_body(nc, tc, pools, xu_tiles, w_ext, ones_col, ident16, ident128,
                        out_ext, dbg_ext)

    nc.compile()
    return nc


def _build_body(nc, tc, pools, xu_tiles, w_ext, ones_col, ident16, ident128,
                out_ext, dbg_ext):
    sb, sbt, ps, dram = pools['sb'], pools['sbt'], pools['ps'], pools['dram']

    def mm_pool(shape, tag="mm", bufs=2):
        return ps.tile(shape, F32, name=tag, tag=tag, bufs=bufs)

    def sq_chunk(src_ap):
        """f32r square of a [128,512] slice (rounding producer for ones-matmul)."""
        q = sbt.tile([128, 512], F32R, name="sqc", tag="sqc", bufs=3)
        nc.vector.tensor_mul(q[:], src_ap, src_ap)
        return q

    SL = [slice(i * 512, (i + 1) * 512) for i in range(SB)]

    # ============ LayerNorm (per-partition stats, per s-block chains) ============
    xn = {}
    for ec in range(EC):
        for sbk in range(SB):
            xn[ec, sbk] = sb.tile([128, 512], F32R, name=f"xn_{ec}_{sbk}")

    from contextlib import nullcontext
    for sbk in range(SB):
        _prio = tc.high_priority() if sbk == 0 else nullcontext()
        _prio.__enter__()
        xu = xu_tiles[sbk]
        st6 = sbt.tile([128, 4, 6], F32, name="st6", tag="st6", bufs=2)
        mv = sbt.tile([128, 4, 2], F32, name="mv", tag="mv", bufs=2)
        inv4 = sbt.tile([128, 4], F32, name="inv4", tag="inv4", bufs=2)
        va = sbt.tile([128, 4], F32, name="va", tag="va", bufs=2)
        vb = sbt.tile([128, 4], F32, name="vb", tag="vb", bufs=2)
        for j in range(4):
            nc.vector.bn_stats(st6[:, j], xu[:, j * E:(j + 1) * E])
            nc.vector.bn_aggr(mv[:, j], st6[:, j])
        nc.vector.tensor_scalar_add(vb[:], mv[:, :, 1], EPS)   # var + eps
        # rsqrt(v) without ACT: v is concentrated near 1 (var of 256-sample
        # LN), so y0 = (1 + 1/v)/2 ~ 1/sqrt(v) to 2nd order; 3 Newton steps
        # take worst-case |v-1| ~ 0.5 to < 1e-6 relative.
        with nc.allow_low_precision("newton-polished below"):
            nc.vector.reciprocal(inv4[:], vb[:])
        nc.vector.tensor_scalar(inv4[:], inv4[:], 0.5, 0.5, OP.mult, OP.add)
        for _ in range(3):
            nc.vector.tensor_mul(va[:], inv4[:], inv4[:])
            nc.vector.tensor_mul(va[:], va[:], vb[:])
            nc.vector.tensor_scalar(va[:], va[:], -0.5, 1.5, OP.mult, OP.add)
            nc.vector.tensor_mul(inv4[:], inv4[:], va[:])
        for j in range(4):
            xnu = sbt.tile([128, E], F32, name="xnu", tag="xnu", bufs=3)
            nc.vector.tensor_scalar(xnu[:], xu[:, j * E:(j + 1) * E],
                                    mv[:, j, 0:1], inv4[:, j:j + 1],
                                    OP.subtract, OP.mult)
            for ec in range(EC):
                pt = mm_pool([128, 128], tag="mmv", bufs=2)
                nc.tensor.transpose(pt[:], xnu[:, ec * 128:(ec + 1) * 128],
                                    ident128[:])
                if ec == 0:
                    nc.scalar.copy(xn[ec, sbk][:, j * 128:(j + 1) * 128], pt[:])
                else:
                    nc.vector.tensor_copy(xn[ec, sbk][:, j * 128:(j + 1) * 128], pt[:])
        _prio.__exit__(None, None, None)

    if dbg_ext:
        for ec in range(EC):
            for sbk in range(SB):
                nc.sync.dma_start(dbg_ext['xn'][ec * 128:(ec + 1) * 128, SL[sbk]],
                                  xn[ec, sbk][:].bitcast(F32))

    def xn_col(ec, st):
        sbk, j = divmod(st, 4)
        return xn[ec, sbk][:, j * 128:(j + 1) * 128]

    # ============ per-head attention ============
    acc = sb.tile([128, ST * E], F32, name="acc")
    if N_HEADS_BUILD == 0:
        nc.any.memset(acc[:], 0.0)

    # two half-bounce tiles: the first AllReduce fires as soon as the last
    # head's W_o finishes s-tiles 0..7, overlapping the remaining compute
    bounce_in = [dram.tile([S // 2, E], F32, name=f"bounce_in{i}",
                           tag=f"bin{i}", bufs=1) for i in range(2)]
    bounce_view = [b.rearrange("(t p) e -> p t e", p=128) for b in bounce_in]

    # Per-head state; emission is software-pipelined across heads so head
    # h+1's (DVE-heavy) projection copies overlap head h's (PE/ACT-heavy)
    # main loop.  Slot grants within a pool tag are FIFO in emission order,
    # so interleaved emission is what actually enables the overlap.
    st_h = {}

    def proj_block(h, wname, ft, sbk, tag, bufs):
        wr = st_h[h]['w'][wname]
        pp = mm_pool([128, 512])
        for ec in range(EC):
            o = ec * E + ft * 128
            nc.tensor.matmul(pp[:], wr[:, o:o + 128], xn[ec, sbk][:],
                             start=(ec == 0), stop=(ec == EC - 1))
        t = sbt.tile([128, 512], F32R, name=tag, tag=tag, bufs=bufs)
        if h == 0 and wname in ("wk", "wq"):
            nc.scalar.copy(t[:], pp[:])   # ACT is exp-free before head 0's main
        else:
            nc.vector.tensor_copy(t[:], pp[:])
        return t

    def row_quarter(tiles_by_ft, sbk, rowdr, h=1):
        rps = mm_pool([1, 512])
        for ft in range(EC):
            sq = sq_chunk(tiles_by_ft[ft][:].bitcast(F32))
            nc.tensor.matmul(rps[:], ones_col[:], sq[:],
                             start=(ft == 0), stop=(ft == EC - 1))
        row = sbt.tile([1, 512], F32, name="rowq", tag="rowq", bufs=2)
        if h == 0:
            nc.scalar.copy(row[:], rps[:])
        else:
            nc.vector.tensor_copy(row[:], rps[:])
        nc.sync.dma_start(rowdr[:, SL[sbk]], row[:])

    def cols_quarter(rowdr, sbk, is_exp, h):
        """[1,512] DRAM quarter -> [128, 4] per-partition cols (no all-K
        barrier: each quarter's bias is ready right after its own K block)."""
        t4 = sbt.tile([4, 128], F32, name="t4", tag="t4", bufs=3)
        nc.sync.dma_start(
            t4[:], rowdr[:, SL[sbk]].rearrange("a (t p) -> (a t) p", p=128))
        pst = ps.tile([128, 4], F32, name="pst", tag="mmv", bufs=2)
        nc.tensor.transpose(pst[:], t4[:], ident16[:4, :4])
        colsq = sbt.tile([128, 4], F32, name="colsq",
                         tag="biasq" if not is_exp else "eq2q", bufs=6)
        if is_exp:
            nc.scalar.activation(colsq[:], pst[:], AF.Exp, scale=-0.5)
        elif h == 0:
            nc.scalar.activation(colsq[:], pst[:], AF.Identity, scale=-0.5)
        else:
            nc.vector.tensor_scalar_mul(colsq[:], pst[:], -0.5)
        return colsq

    def emit_proj(h, sbk):
        """K and Q projection blocks + row quarters + bias quarters."""
        s = st_h[h]
        for ft in range(EC):
            s['kt'][ft, sbk] = proj_block(h, "wk", ft, sbk, "kt", 16)
        row_quarter([s['kt'][ft, sbk] for ft in range(EC)], sbk, s['k2dr'], h)
        s['biasq'][sbk] = cols_quarter(s['k2dr'], sbk, is_exp=False, h=h)
        for ft in range(EC):
            s['qt'][ft, sbk] = proj_block(h, "wq", ft, sbk, "qt", 16)
        row_quarter([s['qt'][ft, sbk] for ft in range(EC)], sbk, s['q2dr'], h)
        s['eq2q'][sbk] = cols_quarter(s['q2dr'], sbk, is_exp=True, h=h)

    def emit_bias(h):
        pass

    def emit_v(h, sbk):
        s = st_h[h]
        wv = s['w']['wv']
        for st in range(sbk * 4, sbk * 4 + 4):
            pv = mm_pool([128, E], tag="mmv", bufs=2)
            for ec in range(EC):
                nc.tensor.matmul(pv[:], xn_col(ec, st),
                                 wv[:, ec * E:(ec + 1) * E],
                                 start=(ec == 0), stop=(ec == EC - 1))
            v = sbt.tile([128, E], F32R, name="vt", tag="vt", bufs=22)
            if st % 2 == 0:
                nc.scalar.copy(v[:], pv[:])
            else:
                nc.vector.tensor_copy(v[:], pv[:])
            s['vt'][st] = v

    def emit_main(h, sbk):
        s = st_h[h]
        kt, qt, vt, biasq = s['kt'], s['qt'], s['vt'], s['biasq']

        def kt_col(ft, tt):
            tb, j = divmod(tt, 4)
            return kt[ft, tb][:, j * 128:(j + 1) * 128]

        ops = [ps.tile([128, 512], F32, name="ovps", tag=f"ovps{ft}", bufs=1)
               for ft in range(EC)]
        sc_q = {}
        SKEW = 3
        for tt in range(ST + SKEW):
            if tt < ST:
                stps = mm_pool([128, 512], tag="stps", bufs=2)
                for ft in range(EC):
                    nc.tensor.matmul(stps[:], kt_col(ft, tt), qt[ft, sbk][:],
                                     start=(ft == 0), stop=(ft == EC - 1))
                sc = sbt.tile([128, 512], F32R, name="sc", tag="sc", bufs=6)
                tb, tj = divmod(tt, 4)
                nc.scalar.activation(sc[:], stps[:], AF.Exp,
                                     bias=biasq[tb][:, tj:tj + 1], scale=1.0)
                sc_q[tt] = sc
            if tt >= SKEW:
                pv_tt = tt - SKEW
                sc_prev = sc_q.pop(pv_tt)
                for ft in range(EC):
                    nc.tensor.matmul(ops[ft][:],
                                     vt[pv_tt][:, ft * 128:(ft + 1) * 128],
                                     sc_prev[:],
                                     start=(pv_tt == 0), stop=(pv_tt == ST - 1))
        for ft in range(EC):
            o = sbt.tile([128, 512], F32R, name="outT", tag="outT", bufs=8)
            if ft == 0:
                nc.scalar.copy(o[:], ops[ft][:])
            else:
                nc.vector.tensor_copy(o[:], ops[ft][:])
            s['outT'][ft, sbk] = o

    def emit_wo(h, sbk):
        s = st_h[h]
        wo = s['w']['wo']
        for st in range(sbk * 4, sbk * 4 + 4):
            j = st % 4
            wops = mm_pool([128, E], tag="mmv", bufs=2)
            for ft in range(EC):
                nc.tensor.matmul(wops[:], s['outT'][ft, sbk][:, j * 128:(j + 1) * 128],
                                 wo[:, ft * E:(ft + 1) * E],
                                 start=(ft == 0), stop=(ft == EC - 1))
            asl = acc[:, st * E:(st + 1) * E]
            qb, qj = divmod(st, 4)
            eqcol = s['eq2q'][qb][:, qj:qj + 1]
            if h == 0:
                nc.vector.tensor_scalar(asl, wops[:], eqcol, None, OP.mult)
            else:
                nc.vector.scalar_tensor_tensor(asl, wops[:], eqcol,
                                               asl, OP.mult, OP.add)
        if h == N_HEADS_BUILD - 1:
            # one batched 3-D DMA per s-block (4 tiles), not 4 setups
            half, sth = divmod(sbk * 4, 8)
            nc.sync.dma_start(
                bounce_view[half][:, sth:sth + 4, :],
                acc[:, sbk * 4 * E:(sbk + 1) * 4 * E]
                .rearrange("p (t e) -> p t e", e=E))

    def new_head_state(h):
        w = {}
        for wname in ("wk", "wq", "wv", "wo"):
            wtmp = sbt.tile([128, EC * E], F32, name="wtmp", tag="wtmp", bufs=3)
            nc.sync.dma_start(wtmp[:], w_ext[wname][h])
            wr = sbt.tile([128, EC * E], F32R, name=f"w_{wname}",
                          tag=f"w_{wname}", bufs=2)
            if h == 0 and wname in ("wk", "wq"):
                nc.scalar.copy(wr[:], wtmp[:])
            else:
                nc.vector.tensor_copy(wr[:], wtmp[:])
            w[wname] = wr
        st_h[h] = dict(w=w, kt={}, qt={}, vt={}, outT={}, biasq={}, eq2q={},
                       k2dr=dram.tile([1, S], F32, name="k2dr", tag="k2dr", bufs=2),
                       q2dr=dram.tile([1, S], F32, name="q2dr", tag="q2dr", bufs=2))

    if N_HEADS_BUILD > 0:
        # head 0: emit everything up front (overlaps LN + loads)
        new_head_state(0)
        for sbk in range(SB):
            emit_proj(0, sbk)
            emit_v(0, sbk)
        emit_bias(0)

    for h in range(N_HEADS_BUILD):
        nxt = h + 1
        if nxt < N_HEADS_BUILD:
            new_head_state(nxt)
        for sbk in range(SB):
            emit_main(h, sbk)
            emit_wo(h, sbk)
            if nxt < N_HEADS_BUILD:
                # front-load the next head's projections: all K/Q blocks can
                # take slots immediately (bufs=16 covers two heads), so the
                # bias chain completes well before main(h) drains.
                if sbk == 0:
                    emit_proj(nxt, 0)
                    emit_proj(nxt, 1)
                elif sbk == 1:
                    emit_proj(nxt, 2)
                    emit_proj(nxt, 3)
                elif sbk == 2:
                    emit_bias(nxt)
                    emit_v(nxt, 0)
                    emit_v(nxt, 1)
                else:
                    emit_v(nxt, 2)
                    emit_v(nxt, 3)

        if dbg_ext and h == 0:
            s = st_h[0]
            for ft in range(EC):
                for sbk in range(SB):
                    nc.sync.dma_start(dbg_ext['qt'][ft * 128:(ft + 1) * 128, SL[sbk]],
                                      s['qt'][ft, sbk][:].bitcast(F32))
            for st in range(ST):
                nc.sync.dma_start(dbg_ext['v'][:, st * E:(st + 1) * E],
                                  s['vt'][st][:].bitcast(F32))
            for qb in range(SB):
                nc.sync.dma_start(dbg_ext['q2'][:, qb * 4:(qb + 1) * 4],
                                  s['eq2q'][qb][:])
        if h > 0:
            st_h.pop(h - 1, None)

    if dbg_ext:
        nc.sync.dma_start(dbg_ext['part'][:], acc[:])

    if N_HEADS_BUILD == 0:
        for half in range(2):
            nc.sync.dma_start(
                bounce_view[half][:, :, :],
                acc[:, half * 8 * E:(half + 1) * 8 * E]
                .rearrange("p (t e) -> p t e", e=E))

    # ============ AllReduce over batch pair + store (two halves) ============
    for half in range(2):
        osl = out_ext[half * (S // 2):(half + 1) * (S // 2), :]
        if NO_COLL:
            nc.sync.dma_start(osl, bounce_in[half][:, :])
        else:
            bo = dram.tile([S // 2, E], F32, name=f"bounce_out{half}",
                           tag=f"bout{half}", bufs=1)
            nc.gpsimd.collective_compute(
                "AllReduce", OP.add,
                replica_groups=[[0, 1], [2, 3], [4, 5], [6, 7]],
                ins=[bounce_in[half].opt()],
                outs=[bo.opt()],
            )
            nc.sync.dma_start(osl, bo[:, :])


# ================= host side =================

def prep_inputs(x, ln_scale, W_q, W_k, W_v, W_o, gamma):
    """Build per-core input maps."""
    x = np.asarray(x, np.float32)
    ln_scale = np.asarray(ln_scale, np.float32)
    W_q = np.asarray(W_q, np.float32)
    W_k = np.asarray(W_k, np.float32)
    W_v = np.asarray(W_v, np.float32)
    W_o = np.asarray(W_o, np.float32)
    gamma = np.asarray(gamma, np.float32).reshape(H)

    in_maps = []
    for c in range(N_CORES):
        b = c // 2
        h0 = HL * (c % 2)
        hs = list(range(h0, h0 + HL))
        g = gamma[hs]
        s2g = np.sqrt(2.0 * g).astype(np.float32)
        wq = (W_q[hs] * ln_scale[None, :, None] * s2g[:, None, None])
        wk = (W_k[hs] * ln_scale[None, :, None] * s2g[:, None, None])
        wv = (W_v[hs] * ln_scale[None, :, None])
        def _lay(w):   # [HL, E_in(=EC*128), E] -> [HL, 128, EC*E]
            return np.ascontiguousarray(
                w.reshape(HL, EC, 128, E).transpose(0, 2, 1, 3).reshape(HL, 128, EC * E))
        wq = _lay(wq)
        wk = _lay(wk)
        wv = _lay(wv)
        wo = _lay(np.stack([W_o[:, 256 * h:256 * (h + 1)].T.copy() for h in hs]))
        in_maps.append({
            "x": np.ascontiguousarray(x[b]),
            "wq": np.ascontiguousarray(wq),
            "wk": np.ascontiguousarray(wk),
            "wv": np.ascontiguousarray(wv),
            "wo": np.ascontiguousarray(wo),
        })
    return in_maps


def assemble_output(results):
    out = np.empty((B, S, E), np.float32)
    for b in range(B):
        out[b] = results[2 * b]["out"]
    return out


_NC_CACHE = {}


def _get_nc():
    if 'nc' not in _NC_CACHE:
        _NC_CACHE['nc'] = build_kernel(R=1, debug=False)
    return _NC_CACHE['nc']


def kernel(x, e=None, p=None, ln_scale=None, W_q=None, W_k=None, W_v=None,
           W_o=None, gamma=None, **_unused):
    """Full-input entry point. e and p are unused by the reference network
    (use_ppe=False config); they are accepted and ignored."""
    in_maps = prep_inputs(x, ln_scale, W_q, W_k, W_v, W_o, gamma)
    nc = _get_nc()
    res = run_bass_kernel_spmd(nc, in_maps, core_ids=list(range(N_CORES)))
    return assemble_output(res.results)



# revision 8
# speedup vs baseline: 1.0841x; 1.0841x over previous
"""RBF-kernel attention (nn_Attention_76081050682051) on 8 TRN2 NeuronCores.

Self-contained Bass/Tile kernel. `kernel(**inputs)` takes the FULL unsharded
inputs of reference.setup_inputs() and returns the FULL [4, 2048, 256] f32
output.

Sharding (B x tensor-parallel heads): core c -> batch b = c//2, heads
[4*(c%2), 4*(c%2)+4); pairwise AllReduce ([0,1],[2,3],[4,5],[6,7]) combines
the two half-head partial outputs of each batch after the W_o projection.

Device math:
  LayerNorm per-partition stats via bn_stats/bn_aggr; rsqrt via DVE
  reciprocal_approx_fast + one ACT Sqrt. xnT built by PE transposes batched
  4-per-psum-tile, drained with one [128,512] copy each.
  Per head: K'T/Q'T = (folded W).T @ xnT in f32r, then quantized to fp8e4
  hi+lo pairs (ACT Identity copy for hi, DVE subtract for lo).  The QK^T
  scores matmul runs as 3 fp8 DoubleRow matmuls per [128,512] tile
  (hi*hi + hi*lo + lo*hi, contraction 256 per instr at 0.5 cyc/row) - 1.33x
  the f32r rate; the dropped lo*lo term and fp8 quantization contribute
  ~0.4% relative error (budget 2e-2).
  k2/q2 row sums: per-feature squares (ACT/DVE Square of the proj psum) then
  [128,1]-output plain-fp32 ones-matmuls put k2/q2 directly on partitions
  (no DRAM roundtrip, ~2ns each vs 213ns [1,512] f32r row matmuls).
  scoresT[t,s] = exp(qk'[t,s] - k2'[t]/2) via one ACT exp per [128,512] tile
  (per-partition bias); exp(-q2'[s]/2) is applied after W_o as a
  per-partition scale.  outT = V.T @ scoresT accumulates over t in f32r
  (fp8 would add ~2.6% error - over budget); W_o runs on outT column slices;
  partial outputs AllReduce within each batch pair.
  Emission is software-pipelined across heads; LayerNorm interleaves with
  head-0 K projections so the first main loop starts as soon as K/k2 of all
  four s-blocks are quantized.
"""
import sys
sys.path.insert(0, '/opt/trn_rl_repo')
import numpy as np
from concourse import bass, bacc, tile, mybir, masks
from concourse.bass_utils import run_bass_kernel_spmd

F32 = mybir.dt.float32
F32R = mybir.dt.float32r
FP8 = mybir.dt.float8e4
AF = mybir.ActivationFunctionType
OP = mybir.AluOpType
DR = mybir.MatmulPerfMode.DoubleRow

B, S, E, H = 4, 2048, 256, 8
HL = 4          # heads per core
EC = 2          # e chunks of 128
SB = 4          # s blocks of 512
ST = 16         # s/t tiles of 128
N_CORES = 8
EPS = 1e-5

NO_COLL = False
N_HEADS_BUILD = HL


def build_kernel(R=1, debug=False):
    nc = bacc.Bacc("TRN2", target_bir_lowering=False, debug=False,
                   num_devices=N_CORES)

    x_ext = nc.declare_dram_parameter("x", [S, E], F32, isOutput=False)
    w_ext = {}
    for wname in ("wq", "wk", "wv", "wo"):
        w_ext[wname] = nc.declare_dram_parameter(wname, [HL, 128, EC * E], F32,
                                                 isOutput=False)
    out_ext = nc.declare_dram_parameter("out", [S, E], F32, isOutput=True)

    with tile.TileContext(nc) as tc:
        with tc.tile_pool(name="sb", bufs=1) as sb, \
             tc.tile_pool(name="sbt", bufs=1) as sbt, \
             tc.tile_pool(name="ps", bufs=1, space="PSUM") as ps, \
             tc.tile_pool(name="dram", bufs=1, space="DRAM") as dram:

            ones32 = sb.tile([128, 1], F32, name="ones32")
            nc.any.memset(ones32[:], 1.0)
            ident128 = sb.tile([128, 128], F32, name="ident128")
            masks.make_identity(nc, ident128[:])

            xu_tiles = []
            for sbk in range(SB):
                xu = sbt.tile([128, 4 * E], F32, name="xu", tag="xu", bufs=4)
                nc.sync.dma_start(
                    xu[:].rearrange("p (t e) -> p t e", t=4),
                    x_ext[sbk * 512:(sbk + 1) * 512, :]
                    .rearrange("(t p) e -> p t e", p=128))
                xu_tiles.append(xu)

            pools = dict(sb=sb, sbt=sbt, ps=ps, dram=dram)
            _build_body(nc, tc, pools, xu_tiles, w_ext, ones32, ident128,
                        out_ext)

    nc.compile()
    return nc


def _build_body(nc, tc, pools, xu_tiles, w_ext, ones32, ident128, out_ext):
    sb, sbt, ps, dram = pools['sb'], pools['sbt'], pools['ps'], pools['dram']

    def big_ps(tag="pp", bufs=2):
        return ps.tile([128, 512], F32, name=tag, tag=tag, bufs=bufs)

    def sm_ps():
        # shared small-psum ring: pstk/pstq/pv/wops all [128, 256]
        return ps.tile([128, 256], F32, name="sm", tag="sm", bufs=2)

    # ============ LayerNorm pieces (called per s-block) ============
    xn = {}
    for ec in range(EC):
        for sbk in range(SB):
            xn[ec, sbk] = sb.tile([128, 512], F32R, name=f"xn_{ec}_{sbk}")

    def emit_ln(sbk):
        xu = xu_tiles[sbk]
        st6 = sbt.tile([128, 4, 6], F32, name="st6", tag="st6", bufs=2)
        mv = sbt.tile([128, 4, 2], F32, name="mv", tag="mv", bufs=2)
        vb = sbt.tile([128, 4], F32, name="vb", tag="vb", bufs=2)
        rcp = sbt.tile([128, 4], F32, name="rcp", tag="rcp", bufs=2)
        inv4 = sbt.tile([128, 4], F32, name="inv4", tag="inv4", bufs=2)
        for j in range(4):
            nc.vector.bn_stats(st6[:, j], xu[:, j * E:(j + 1) * E])
            nc.vector.bn_aggr(mv[:, j], st6[:, j])
        nc.vector.tensor_scalar_add(vb[:], mv[:, :, 1], EPS)
        with nc.allow_low_precision("~18-bit reciprocal + table sqrt is"
                                    " plenty for a LN scale"):
            nc.vector.reciprocal_approx_fast(rcp[:], vb[:])
        nc.scalar.activation(inv4[:], rcp[:], AF.Sqrt, scale=1.0)
        xnus = []
        for j in range(4):
            xnu = sbt.tile([128, E], F32, name="xnu", tag="xnu", bufs=6)
            nc.vector.tensor_scalar(xnu[:], xu[:, j * E:(j + 1) * E],
                                    mv[:, j, 0:1], inv4[:, j:j + 1],
                                    OP.subtract, OP.mult)
            xnus.append(xnu)
        for ec in range(EC):
            pt2 = big_ps()
            for j in range(4):
                nc.tensor.transpose(pt2[:, j * 128:(j + 1) * 128],
                                    xnus[j][:, ec * 128:(ec + 1) * 128],
                                    ident128[:])
            if ec == 0:
                nc.scalar.copy(xn[ec, sbk][:], pt2[:])
            else:
                nc.vector.tensor_copy(xn[ec, sbk][:], pt2[:])

    def xn_col(ec, st):
        sbk, j = divmod(st, 4)
        return xn[ec, sbk][:, j * 128:(j + 1) * 128]

    SL = [slice(i * 512, (i + 1) * 512) for i in range(SB)]

    # ============ per-head state ============
    acc = sb.tile([128, ST * E], F32, name="acc")

    bounce_in = [dram.tile([S // 2, E], F32, name=f"bounce_in{i}",
                           tag=f"bin{i}", bufs=1) for i in range(2)]
    bounce_view = [b.rearrange("(t p) e -> p t e", p=128) for b in bounce_in]

    st_h = {}

    def new_head_state(h):
        w = {}
        for i, wname in enumerate(("wk", "wq", "wv", "wo")):
            wtmp = sbt.tile([128, EC * E], F32, name="wtmp", tag="wtmp",
                            bufs=3)
            nc.sync.dma_start(wtmp[:], w_ext[wname][h])
            wr = sbt.tile([128, EC * E], F32R, name=f"w_{wname}",
                          tag=f"w_{wname}", bufs=2)
            if i % 2 == 0:
                nc.vector.tensor_copy(wr[:], wtmp[:])
            else:
                nc.scalar.copy(wr[:], wtmp[:])
            w[wname] = wr
        st_h[h] = dict(w=w, khi={}, klo={}, qhi={}, qlo={}, vt={}, outT={},
                       biasq={}, eq2q={})

    def emit_proj(h, sbk, which):
        """f32r projection of K^T or Q^T for one s-block + fp8 hi/lo
        quantization + squares + k2/q2 per-partition columns."""
        s = st_h[h]
        wr = s['w']['wk' if which == 'k' else 'wq']
        hi = sbt.tile([128, 2, 512], FP8, name=which + "hi", tag=which + "hi",
                      bufs=8)
        lo = sbt.tile([128, 2, 512], FP8, name=which + "lo", tag=which + "lo",
                      bufs=8)
        sqs = []
        for ft in range(EC):
            pp = big_ps()
            for ec in range(EC):
                o = ec * E + ft * 128
                nc.tensor.matmul(pp[:], wr[:, o:o + 128],
                                 xn[ec, sbk][:],
                                 start=(ec == 0), stop=(ec == EC - 1))
            if h == 0:
                nc.scalar.copy(hi[:, ft, :], pp[:])
            else:
                nc.vector.tensor_copy(hi[:, ft, :], pp[:])
            nc.vector.tensor_tensor(lo[:, ft, :], pp[:], hi[:, ft, :],
                                    OP.subtract)
            sq = sbt.tile([128, 512], F32, name="sq", tag="sq", bufs=4)
            nc.scalar.activation(sq[:], pp[:], AF.Square, scale=1.0)
            sqs.append(sq)
        pst = sm_ps()
        for j in range(4):
            for ft in range(EC):
                nc.tensor.matmul(pst[:, j:j + 1],
                                 sqs[ft][:, j * 128:(j + 1) * 128],
                                 ones32[:], start=(ft == 0),
                                 stop=(ft == EC - 1))
        if which == 'k':
            s['khi'][sbk], s['klo'][sbk] = hi, lo
            bq = sbt.tile([128, 4], F32, name="biasq", tag="biasq", bufs=8)
            nc.vector.tensor_scalar_mul(bq[:], pst[:, 0:4], -0.5)
            s['biasq'][sbk] = bq
        else:
            s['qhi'][sbk], s['qlo'][sbk] = hi, lo
            eq = sbt.tile([128, 4], F32, name="eq2q", tag="eq2q", bufs=8)
            nc.scalar.activation(eq[:], pst[:, 0:4], AF.Exp, scale=-0.5)
            s['eq2q'][sbk] = eq

    def emit_v(h, sbk):
        s = st_h[h]
        wv = s['w']['wv']
        for st in range(sbk * 4, sbk * 4 + 4):
            pv = sm_ps()
            for ec in range(EC):
                nc.tensor.matmul(pv[:], xn_col(ec, st),
                                 wv[:, ec * E:(ec + 1) * E],
                                 start=(ec == 0), stop=(ec == EC - 1))
            v = sbt.tile([128, E], F32R, name="vt", tag="vt", bufs=24)
            nc.vector.tensor_copy(v[:], pv[:])
            s['vt'][st] = v

    def emit_main(h, sbk):
        s = st_h[h]
        khi, klo, qhi, qlo = s['khi'], s['klo'], s['qhi'], s['qlo']
        vt, biasq = s['vt'], s['biasq']
        qh, ql = qhi[sbk][:], qlo[sbk][:]

        ops = [big_ps(tag="ov", bufs=2) for _ in range(EC)]
        sc_q = {}
        SKEW = 3
        for tt in range(ST + SKEW):
            if tt < ST:
                tb, tj = divmod(tt, 4)
                csl = slice(tj * 128, (tj + 1) * 128)
                kh = khi[tb][:, :, csl]
                kl = klo[tb][:, :, csl]
                stps = big_ps(tag="stps", bufs=2)
                nc.tensor.matmul(stps[:], kh, qh, start=True, stop=False,
                                 perf_mode=DR)
                nc.tensor.matmul(stps[:], kh, ql, start=False, stop=False,
                                 perf_mode=DR)
                nc.tensor.matmul(stps[:], kl, qh, start=False, stop=True,
                                 perf_mode=DR)
                sc = sbt.tile([128, 512], F32R, name="sc", tag="sc", bufs=6)
                nc.scalar.activation(sc[:], stps[:], AF.Exp,
                                     bias=biasq[tb][:, tj:tj + 1], scale=1.0)
                sc_q[tt] = sc
            if tt >= SKEW:
                pv_tt = tt - SKEW
                sc_prev = sc_q.pop(pv_tt)
                for ft in range(EC):
                    nc.tensor.matmul(ops[ft][:],
                                     vt[pv_tt][:, ft * 128:(ft + 1) * 128],
                                     sc_prev[:],
                                     start=(pv_tt == 0), stop=(pv_tt == ST - 1))
        for ft in range(EC):
            o = sbt.tile([128, 512], F32R, name="outT", tag="outT", bufs=8)
            if ft == 0:
                nc.scalar.copy(o[:], ops[ft][:])
            else:
                nc.vector.tensor_copy(o[:], ops[ft][:])
            s['outT'][ft, sbk] = o

    def emit_wo(h, sbk):
        s = st_h[h]
        wo = s['w']['wo']
        for st in range(sbk * 4, sbk * 4 + 4):
            j = st % 4
            wops = sm_ps()
            for ft in range(EC):
                nc.tensor.matmul(wops[:],
                                 s['outT'][ft, sbk][:, j * 128:(j + 1) * 128],
                                 wo[:, ft * E:(ft + 1) * E],
                                 start=(ft == 0), stop=(ft == EC - 1))
            asl = acc[:, st * E:(st + 1) * E]
            qb, qj = divmod(st, 4)
            eqcol = s['eq2q'][qb][:, qj:qj + 1]
            if h == 0:
                nc.vector.tensor_scalar(asl, wops[:], eqcol, None, OP.mult)
            else:
                nc.vector.scalar_tensor_tensor(asl, wops[:], eqcol,
                                               asl, OP.mult, OP.add)
        if h == N_HEADS_BUILD - 1:
            half, sth = divmod(sbk * 4, 8)
            nc.sync.dma_start(
                bounce_view[half][:, sth:sth + 4, :],
                acc[:, sbk * 4 * E:(sbk + 1) * 4 * E]
                .rearrange("p (t e) -> p t e", e=E))

    # ============ emission schedule ============
    if N_HEADS_BUILD == 0:
        nc.any.memset(acc[:], 0.0)
    else:
        new_head_state(0)
        # LN interleaved with head-0 K projections: main(0,0) needs K/k2 of
        # all four s-blocks, so those quantization chains are the warmup
        # critical path.
        for sbk in range(SB):
            emit_ln(sbk)
            emit_proj(0, sbk, 'k')
        emit_proj(0, 0, 'q')
        emit_proj(0, 1, 'q')
        for sbk in range(SB):
            emit_v(0, sbk)

    for h in range(N_HEADS_BUILD):
        nxt = h + 1
        if nxt < N_HEADS_BUILD:
            new_head_state(nxt)
        for sbk in range(SB):
            emit_main(h, sbk)
            emit_wo(h, sbk)
            if h == 0:
                # finish head 0's own pieces
                if sbk == 0:
                    emit_proj(0, 2, 'q')
                    emit_proj(0, 3, 'q')
            if nxt < N_HEADS_BUILD:
                if sbk == 0:
                    emit_proj(nxt, 0, 'k')
                    emit_proj(nxt, 1, 'k')
                elif sbk == 1:
                    emit_proj(nxt, 2, 'k')
                    emit_proj(nxt, 3, 'k')
                elif sbk == 2:
                    emit_proj(nxt, 0, 'q')
                    emit_proj(nxt, 1, 'q')
                    emit_v(nxt, 0)
                    emit_v(nxt, 1)
                else:
                    emit_proj(nxt, 2, 'q')
                    emit_proj(nxt, 3, 'q')
                    emit_v(nxt, 2)
                    emit_v(nxt, 3)
        if h > 0:
            st_h.pop(h - 1, None)

    if N_HEADS_BUILD == 0:
        for half in range(2):
            nc.sync.dma_start(
                bounce_view[half][:, :, :],
                acc[:, half * 8 * E:(half + 1) * 8 * E]
                .rearrange("p (t e) -> p t e", e=E))

    # ============ AllReduce over batch pair + store (two halves) ============
    for half in range(2):
        osl = out_ext[half * (S // 2):(half + 1) * (S // 2), :]
        if NO_COLL:
            nc.sync.dma_start(osl, bounce_in[half][:, :])
        else:
            bo = dram.tile([S // 2, E], F32, name=f"bounce_out{half}",
                           tag=f"bout{half}", bufs=1)
            nc.gpsimd.collective_compute(
                "AllReduce", OP.add,
                replica_groups=[[0, 1], [2, 3], [4, 5], [6, 7]],
                ins=[bounce_in[half].opt()],
                outs=[bo.opt()],
            )
            nc.sync.dma_start(osl, bo[:, :])


# ================= host side =================

def prep_inputs(x, ln_scale, W_q, W_k, W_v, W_o, gamma):
    """Build per-core input maps."""
    x = np.asarray(x, np.float32)
    ln_scale = np.asarray(ln_scale, np.float32)
    W_q = np.asarray(W_q, np.float32)
    W_k = np.asarray(W_k, np.float32)
    W_v = np.asarray(W_v, np.float32)
    W_o = np.asarray(W_o, np.float32)
    gamma = np.asarray(gamma, np.float32).reshape(H)

    in_maps = []
    for c in range(N_CORES):
        b = c // 2
        h0 = HL * (c % 2)
        hs = list(range(h0, h0 + HL))
        g = gamma[hs]
        s2g = np.sqrt(2.0 * g).astype(np.float32)
        wq = (W_q[hs] * ln_scale[None, :, None] * s2g[:, None, None])
        wk = (W_k[hs] * ln_scale[None, :, None] * s2g[:, None, None])
        wv = (W_v[hs] * ln_scale[None, :, None])
        def _lay(w):   # [HL, E_in(=EC*128), E] -> [HL, 128, EC*E]
            return np.ascontiguousarray(
                w.reshape(HL, EC, 128, E).transpose(0, 2, 1, 3).reshape(HL, 128, EC * E))
        wq = _lay(wq)
        wk = _lay(wk)
        wv = _lay(wv)
        wo = _lay(np.stack([W_o[:, 256 * h:256 * (h + 1)].T.copy() for h in hs]))
        in_maps.append({
            "x": np.ascontiguousarray(x[b]),
            "wq": np.ascontiguousarray(wq),
            "wk": np.ascontiguousarray(wk),
            "wv": np.ascontiguousarray(wv),
            "wo": np.ascontiguousarray(wo),
        })
    return in_maps


def assemble_output(results):
    out = np.empty((B, S, E), np.float32)
    for b in range(B):
        out[b] = results[2 * b]["out"]
    return out


_NC_CACHE = {}


def _get_nc():
    if 'nc' not in _NC_CACHE:
        _NC_CACHE['nc'] = build_kernel(R=1, debug=False)
    return _NC_CACHE['nc']


def kernel(x, e=None, p=None, ln_scale=None, W_q=None, W_k=None, W_v=None,
           W_o=None, gamma=None, **_unused):
    """Full-input entry point. e and p are unused by the reference network
    (use_ppe=False config); they are accepted and ignored."""
    in_maps = prep_inputs(x, ln_scale, W_q, W_k, W_v, W_o, gamma)
    nc = _get_nc()
    res = run_bass_kernel_spmd(nc, in_maps, core_ids=list(range(N_CORES)))
    return assemble_output(res.results)
